# revision 10
# baseline (speedup 1.0000x reference)
"""Distance-based attention (nn_Attention_67989332296336) on 8 TRN2 NeuronCores.

Math per batch element b (S=1024, E=H=A=256):
    d2[t,j]  = |x_t|^2 + |x_j|^2 - 2 x_t.x_j
    dist     = sqrt(max(d2,0)+eps)
    scores   = w_sim*dist + b_sim
    A        = softmax_j(scores)
    G        = A @ h
    Z        = tanh([G, h] @ W_g^T + b_g)

Sharding: batch dim B=32 split over 8 cores (4 per core), weights replicated.

v2 strategy (vs the phase-split baseline):
  - Symmetry: dist/P are symmetric per batch, so the gram, sqrt and exp
    passes run only on the lower block-triangle (36/64 tiles), stored
    compacted ([128, 4608] per batch).  PV lhsT tiles for k >= i come
    straight out of the compact store; the 28 missing (k < i) tiles are
    128x128 PE transposes of stored tiles (bf16, FWL) copied out of PSUM
    by ScalarE (small groups) / DVE (large groups).
  - |x_t|^2 squares run on ScalarE (Square activation + accum_out),
    freeing DVE; sqrt applies scale=-2 / bias=|x_t|^2+MARGIN per i-tile;
    exp is one [128,4608] instruction per batch (scale=w_sim).
  - ScalarE runs exactly two table sets: all Square/Sqrt (stage 1),
    then all Exp/Copy/Tanh (stage 2), pinned by same-engine deps.
  - Inputs stream per batch (x in 2 chunks, then h), alternating, so
    batch-0 transposes start ~3us in; gram rows i need only x tiles
    0..i.  Gate fold, blockdiag aug matmul, ones-column denominators as
    in the baseline.  PSUM: 2 banks transposes + 4 banks d2 + 2 banks
    hW/PV = exactly 8.
"""

import sys

import numpy as np

if "/opt/trn_rl_repo" not in sys.path:
    sys.path.append("/opt/trn_rl_repo")

import concourse.bacc as bacc
import concourse.bass as bass
import concourse.mybir as mybir
import concourse.tile as tile
from concourse.bass import ts
from concourse.bass_utils import run_bass_kernel_spmd
from concourse.masks import make_identity

F32 = mybir.dt.float32
BF16 = mybir.dt.bfloat16
FP8 = mybir.dt.float8e4
DR = mybir.MatmulPerfMode.DoubleRow
AF = mybir.ActivationFunctionType
OP = mybir.AluOpType

S = 1024
B = 32
NCORES = 8
BS = B // NCORES  # batches per core
E = 256
H = 256
A = 256
NT = S // 128  # 8 t-tiles
MARGIN = 16.0  # replaces max(d2,0)+eps; absorbs fp8 gram error (cancels in softmax)

# lower-triangle compact layout: row-block i holds cols j in [0, 128*(i+1))
NI = [128 * (i + 1) for i in range(NT)]
OFF = [64 * i * (i + 1) for i in range(NT)]  # sum of NI[:i]
TRI = OFF[NT - 1] + NI[NT - 1]  # 4608
LOFF = [64 * i * (i - 1) for i in range(NT)]  # p_lo group offset: 128*(0+1+..+i-1)
NLO = 28 * 128  # 3584


def _chain(prev, cur, reason):
    """Pin same-engine execution order (table-set discipline)."""
    if prev is not None:
        tile.add_dep_helper(cur.ins, prev.ins, sync=False, reason=reason)
    return cur


def build_graph():
    nc = bacc.Bacc("TRN2", target_bir_lowering=False, debug=False)

    x_ext = nc.declare_dram_parameter("x", [S, BS, E], F32, isOutput=False)
    h_ext = nc.declare_dram_parameter("h", [S, BS, H], F32, isOutput=False)
    w_ext = nc.declare_dram_parameter("w_sim", [1, 1], F32, isOutput=False)
    wg_ext = nc.declare_dram_parameter("W_g", [A, 2 * H], F32, isOutput=False)
    bg_ext = nc.declare_dram_parameter("b_g", [1, A], F32, isOutput=False)
    out_ext = nc.declare_dram_parameter("out", [S, BS, A], F32, isOutput=True)

    with tile.TileContext(nc) as tc:
        with (
            tc.tile_pool(name="consts", bufs=1) as consts,
            tc.tile_pool(name="nat", bufs=2) as natp,
            tc.tile_pool(name="xt", bufs=2) as xtp,
            tc.tile_pool(name="small", bufs=2) as smallp,
            tc.tile_pool(name="dcmp", bufs=BS) as dcmpp,
            tc.tile_pool(name="pcmp", bufs=2) as pcmpp,
            tc.tile_pool(name="plo", bufs=2) as plop,
            tc.tile_pool(name="hw", bufs=BS) as hwp,
            tc.tile_pool(name="zt", bufs=2) as ztp,
            tc.tile_pool(name="ps_tr", bufs=2, space="PSUM") as pstr,
            tc.tile_pool(name="ps_big", bufs=2, space="PSUM") as psbig,
        ):
            # ---- PE HAM warm-up: dense burst while input DMAs fly ----
            warm_in = consts.tile([128, 128], BF16)
            nc.vector.memset(warm_in, 1.0)
            warm_ps = psbig.tile([128, 512], F32, tag="big")
            for _ in range(40):
                nc.tensor.matmul(
                    warm_ps[:, 0:128], warm_in[:], warm_in[:], start=True, stop=True
                )

            # ---- input DMAs: stream per batch, x (2 chunks) then h ----
            xnat_list, hnat_list = [], []
            for b in range(BS):
                xnat = natp.tile([128, NT, E], BF16, tag="xnat")
                xnat_list.append(xnat)
                xsrc = x_ext[:, b, :].rearrange("(i p) e -> p i e", p=128)
                cw = 2 if b == 0 else 4
                for g in range(NT // cw):
                    nc.gpsimd.dma_start(
                        out=xnat[:, cw * g : cw * (g + 1), :],
                        in_=xsrc[:, cw * g : cw * (g + 1), :],
                    )
                hnat = natp.tile([128, NT, H], BF16, tag="hnat")
                hnat_list.append(hnat)
                nc.gpsimd.dma_start(
                    out=hnat,
                    in_=h_ext[:, b, :].rearrange("(i p) e -> p i e", p=128),
                )

            # ---------------- constants ----------------
            ident = consts.tile([128, 128], F32)
            make_identity(nc, ident)
            identb = consts.tile([128, 128], BF16)
            nc.vector.tensor_copy(identb, ident)
            ones_stage = consts.tile([8, 128], F32)
            nc.vector.memset(ones_stage, 1.0)
            ones_row = consts.tile([1, 128], BF16)
            nc.vector.tensor_copy(ones_row, ones_stage[0:1, :])
            ones8 = consts.tile([8, 128], BF16)
            nc.vector.tensor_copy(ones8, ones_stage)

            w_col = consts.tile([128, 1], F32)
            nc.sync.dma_start(out=w_col, in_=w_ext[:].partition_broadcast(128))
            bg_stage = consts.tile([1, A], F32)
            nc.sync.dma_start(out=bg_stage, in_=bg_ext[:])
            bg_row = consts.tile([1, A], BF16)
            nc.vector.tensor_copy(bg_row, bg_stage)

            # W_g (A, 2H) -> W12T: 2 k-tiles of (128hd, [A | A]) used as hW rhs
            wnat = consts.tile([128, 2, 2 * H], F32)
            nc.sync.dma_start(
                out=wnat, in_=wg_ext[:].rearrange("(m p) k -> p m k", m=2)
            )
            w12t = consts.tile([128, 2, 2 * H], BF16)
            for k2 in range(2):
                ps = psbig.tile([128, 512], F32, tag="big")
                for w in range(2):
                    for m in range(2):
                        nc.tensor.transpose(
                            ps[:, w * 256 + m * 128 : w * 256 + (m + 1) * 128],
                            wnat[:, m, w * 256 + k2 * 128 : w * 256 + (k2 + 1) * 128],
                            ident[:],
                        )
                nc.vector.tensor_copy(w12t[:, k2, :], ps[:])

            # force the Sqrt table set to load once, up front
            dummy = consts.tile([128, 1], F32)
            sc_prev = nc.scalar.activation(out=dummy, in_=w_col, func=AF.Sqrt)

            # -------- per-batch state --------
            d_cmp = [dcmpp.tile([128, TRI], BF16, tag="d", name=f"dcmp{b}") for b in range(BS)]
            hw_l = [hwp.tile([128, NT, 520], BF16, tag="hw", name=f"hw{b}") for b in range(BS)]

            def transpose_nat(nat, dstT):
                """nat [128, NT, 256] -> dstT [128, 2, S] (k-major)."""
                for g in range(2):
                    psT = pstr.tile([128, 1024], BF16, tag="tr")
                    for i2 in range(4):
                        i = 4 * g + i2
                        for k2 in range(2):
                            nc.tensor.transpose(
                                psT[:, i2 * 256 + k2 * 128 : i2 * 256 + (k2 + 1) * 128],
                                nat[:, i, ts(k2, 128)],
                                identb[:],
                            )
                    nc.vector.tensor_copy(
                        dstT[:, :, 512 * g : 512 * (g + 1)].rearrange(
                            "p k (i f) -> p i k f", i=4
                        ),
                        psT[:].rearrange("p (i k f) -> p i k f", i=4, k=2),
                    )

            def do_hw(b):
                """hW = h @ [W1|W2]^T (+ bg on the W2 half) + ones column."""
                hT = xtp.tile([128, 2, S], BF16, tag="hT")
                transpose_nat(hnat_list[b], hT)
                hw = hw_l[b]
                for m in range(NT):
                    ps = psbig.tile([128, 512], F32, tag="big")
                    nc.tensor.matmul(
                        ps[:], hT[:, 0, ts(m, 128)], w12t[:, 0, :],
                        start=True, stop=False,
                    )
                    nc.tensor.matmul(
                        ps[:], hT[:, 1, ts(m, 128)], w12t[:, 1, :],
                        start=False, stop=False,
                    )
                    nc.tensor.matmul(
                        ps[:, 256:512], ones_row[:], bg_row[:],
                        start=False, stop=True,
                    )
                    hwm = hw[:, m, :]
                    dst = bass.AP(
                        tensor=hwm.tensor,
                        offset=hwm.offset,
                        ap=[hwm.ap[0], [257, 2], [1, 256]],
                    )
                    nc.vector.tensor_copy(dst, ps[:].rearrange("p (u f) -> p u f", u=2))
                nc.vector.memset(hw[:, :, 256:257], 1.0)

            # ================= stage 1: x->dist triangle (+ hW) =================
            stage1 = tc.tile_pool(name="ps_d2", bufs=2, space="PSUM")
            psd2 = stage1.__enter__()
            for b in range(BS):
                xnat = xnat_list[b]
                xT = xtp.tile([128, 2, S], FP8, tag="xT")
                transpose_nat(xnat, xT)

                # per x-chunk g: |x_t|^2 (DVE) -> bias half + blockdiag half
                bias_h = []
                bd_h = []
                for g in range(2):
                    sqm = smallp.tile([128, 4], F32, tag=f"sqm{g}", name=f"sqm{g}")
                    for u in range(4):
                        i = 4 * g + u
                        scr = smallp.tile([128, E], BF16, tag="scr")
                        nc.vector.scalar_tensor_tensor(
                            out=scr,
                            in0=xnat[:, i, :],
                            scalar=1.0,
                            in1=xnat[:, i, :],
                            op0=OP.mult,
                            op1=OP.mult,
                            accum_out=sqm[:, u : u + 1],
                        )
                    biasg = smallp.tile([128, 4], F32, tag=f"bias{g}", name=f"bias{g}")
                    bias_h.append(biasg)
                    nc.vector.tensor_scalar_add(out=biasg, in0=sqm, scalar1=MARGIN)
                    sqmb = smallp.tile([128, 4], BF16, tag=f"sqmb{g}", name=f"sqmb{g}")
                    nc.vector.tensor_copy(sqmb[:], sqm[:])
                    sq4 = pstr.tile([4, 128], BF16, tag="tr", name="sq4")
                    nc.tensor.transpose(sq4[:], sqmb[:], identb[:])
                    sq4sb = smallp.tile([4, 128], BF16, tag=f"sq4sb{g}", name=f"sq4sb{g}")
                    nc.vector.tensor_scalar_mul(sq4sb[:], sq4[:], -0.5)
                    bdg = smallp.tile([8, 512], BF16, tag=f"bd{g}", name=f"bd{g}")
                    bd_h.append(bdg)
                    nc.vector.memset(bdg, 0.0)
                    bd = bdg[:]
                    diag_view = bass.AP(
                        tensor=bd.tensor, offset=bd.offset, ap=[[512 + 128, 4], [1, 128]]
                    )
                    nc.sync.dma_start(out=diag_view, in_=sq4sb[:])

                # gram rows (lower triangle) -> d2 psum -> sqrt -> compact dist
                for i in range(NT):
                    ni = NI[i]
                    d2 = psd2.tile([128, 1024], F32, tag="d2")
                    nchunk = (ni + 511) // 512
                    for c in range(nchunk):
                        j0 = 512 * c
                        j1 = min(ni, j0 + 512)
                        nc.tensor.matmul(
                            d2[:, j0:j1],
                            xT[:, :, ts(i, 128)],
                            xT[:, :, j0:j1],
                            start=True,
                            stop=False,
                            perf_mode=DR,
                        )
                        nc.tensor.matmul(
                            d2[:, j0:j1],
                            ones8[:],
                            bd_h[c][:, 0 : j1 - j0],
                            start=False,
                            stop=True,
                        )
                    si = nc.scalar.activation(
                        out=d_cmp[b][:, OFF[i] : OFF[i] + ni],
                        in_=d2[:, 0:ni],
                        func=AF.Sqrt,
                        bias=bias_h[i // 4][:, i % 4 : i % 4 + 1],
                        scale=-2.0,
                    )
                    sc_prev = _chain(sc_prev, si, "act-order")

                if b < 2:
                    do_hw(b)
            do_hw(2)
            do_hw(3)
            stage1.__exit__(None, None, None)
            stage2 = tc.tile_pool(name="ps_pv", bufs=4, space="PSUM")
            pspv = stage2.__enter__()

            # ================= stage 2: exp, PV, gate =================
            sc_box = [sc_prev]
            p_cmp_l = {}

            def emit_exp(b):
                if b in p_cmp_l:
                    return
                pc = pcmpp.tile([128, TRI], BF16, tag="p", name=f"pcmp{b}")
                ei = nc.scalar.activation(
                    out=pc, in_=d_cmp[b], func=AF.Exp, scale=w_col[:, 0:1]
                )
                sc_box[0] = _chain(sc_box[0], ei, "act-order")
                p_cmp_l[b] = pc

            emit_exp(0)
            for b in range(BS):
                p_cmp = p_cmp_l[b]
                p_lo = plop.tile([128, NLO], BF16, tag="plo", name=f"plo{b}")
                hw = hw_l[b]

                def emit_transp(i):
                    # transpose the missing lhsT tiles (k < i) for PV_i
                    psT = pstr.tile([128, 1024], BF16, tag="tr")
                    for k in range(i):
                        nc.tensor.transpose(
                            psT[:, ts(k, 128)],
                            p_cmp[:, OFF[i] + 128 * k : OFF[i] + 128 * (k + 1)],
                            identb[:],
                        )
                    dst = p_lo[:, LOFF[i] : LOFF[i] + 128 * i]
                    if i <= 3:
                        ci = nc.scalar.copy(dst, psT[:, 0 : 128 * i])
                        sc_box[0] = _chain(sc_box[0], ci, "act-order")
                    else:
                        nc.vector.tensor_copy(dst, psT[:, 0 : 128 * i])

                zs = None
                for i in range(NT):
                    if i + 1 < NT:
                        emit_transp(i + 1)
                    if i == 1 and b + 1 < BS:
                        emit_exp(b + 1)
                    if i % 4 == 0:
                        zs = ztp.tile([128, 4, A], F32, tag="zs", bufs=3)
                    pv = pspv.tile([128, 512], F32, tag="pv")
                    for k in range(NT):
                        if k >= i:
                            lhsT = p_cmp[:, OFF[k] + 128 * i : OFF[k] + 128 * (i + 1)]
                        else:
                            lhsT = p_lo[:, LOFF[i] + 128 * k : LOFF[i] + 128 * (k + 1)]
                        nc.tensor.matmul(
                            pv[:, 0 : A + 1],
                            lhsT,
                            hw[:, k, 0 : A + 1],
                            start=(k == 0),
                            stop=(k == NT - 1),
                        )
                    rp_i = smallp.tile([128, 1], F32, tag="rp_i")
                    nc.vector.reciprocal(rp_i[:], pv[:, A : A + 1])
                    nc.vector.scalar_tensor_tensor(
                        out=zs[:, i % 4, :],
                        in0=pv[:, 0:A],
                        scalar=rp_i[:, 0:1],
                        in1=hw[:, i, 257 : 257 + A],
                        op0=OP.mult,
                        op1=OP.add,
                    )
                    if i % 4 == 3:
                        g2 = i // 4
                        zo = ztp.tile([128, 4, A], F32, tag="zo")
                        ti = nc.scalar.activation(
                            out=zo[:].rearrange("p a b -> p (a b)"),
                            in_=zs[:].rearrange("p a b -> p (a b)"),
                            func=AF.Tanh,
                        )
                        sc_box[0] = _chain(sc_box[0], ti, "act-order")
                        for q in range(2):
                            r0 = 512 * g2 + 256 * q
                            nc.gpsimd.dma_start(
                                out=out_ext[r0 : r0 + 256, b, :].rearrange(
                                    "(u p) a -> p u a", p=128
                                ),
                                in_=zo[:, 2 * q : 2 * q + 2, :],
                            )
            stage2.__exit__(None, None, None)

    nc.compile()
    return nc


_CACHED = {}


def _get_graph():
    if "nc" not in _CACHED:
        _CACHED["nc"] = build_graph()
    return _CACHED["nc"]


def _run(inputs, trace=False, **kw):
    nc = _get_graph()
    x = np.asarray(inputs["x"], dtype=np.float32)
    h = np.asarray(inputs["h"], dtype=np.float32)
    w_sim = np.asarray(inputs["w_sim"], dtype=np.float32).reshape(1, 1)
    W_g = np.ascontiguousarray(np.asarray(inputs["W_g"], dtype=np.float32))
    b_g = np.asarray(inputs["b_g"], dtype=np.float32).reshape(1, A)
    in_maps = []
    for c in range(NCORES):
        in_maps.append(
            {
                "x": np.ascontiguousarray(x[:, c * BS : (c + 1) * BS, :]),
                "h": np.ascontiguousarray(h[:, c * BS : (c + 1) * BS, :]),
                "w_sim": w_sim,
                "W_g": W_g,
                "b_g": b_g,
            }
        )
    res = run_bass_kernel_spmd(nc, in_maps, list(range(NCORES)), trace=trace, **kw)
    out = np.concatenate([res.results[c]["out"] for c in range(NCORES)], axis=1)
    return out, res


def kernel(**inputs):
    out, _ = _run(inputs, trace=False)
    return out


if __name__ == "__main__":
    rng = np.random.default_rng(0)
    ins = {
        "x": rng.standard_normal((S, B, E), dtype=np.float32),
        "h": rng.standard_normal((S, B, H), dtype=np.float32),
        "w_sim": np.array([0.03], dtype=np.float32),
        "b_sim": np.array([0.01], dtype=np.float32),
        "W_g": (rng.standard_normal((A, 2 * H)) * 0.05).astype(np.float32),
        "b_g": np.zeros(A, dtype=np.float32),
    }
    out = kernel(**ins)
    print("out", out.shape, out.dtype, np.abs(out).mean())


# revision 11
# speedup vs baseline: 1.1955x; 1.1955x over previous
"""Distance-based attention (nn_Attention_67989332296336) on 8 TRN2 NeuronCores.

Math per batch element b (S=1024, E=H=A=256):
    d2[t,j]  = |x_t|^2 + |x_j|^2 - 2 x_t.x_j
    dist     = sqrt(max(d2,0)+eps)
    scores   = w_sim*dist + b_sim
    A        = softmax_j(scores)
    G        = A @ h
    Z        = tanh([G, h] @ W_g^T + b_g)

Sharding: batch dim B=32 split over 8 cores (4 per core), weights replicated.

v2 strategy (vs the phase-split baseline):
  - Symmetry: dist/P are symmetric per batch, so the gram, sqrt and exp
    passes run only on the lower block-triangle (36/64 tiles), stored
    compacted ([128, 4608] per batch).  PV lhsT tiles for k >= i come
    straight out of the compact store; the 28 missing (k < i) tiles are
    128x128 PE transposes of stored tiles (bf16, FWL) copied out of PSUM
    by ScalarE (small groups) / DVE (large groups).
  - |x_t|^2 squares run on ScalarE (Square activation + accum_out),
    freeing DVE; sqrt applies scale=-2 / bias=|x_t|^2+MARGIN per i-tile;
    exp is one [128,4608] instruction per batch (scale=w_sim).
  - ScalarE runs exactly two table sets: all Square/Sqrt (stage 1),
    then all Exp/Copy/Tanh (stage 2), pinned by same-engine deps.
  - Inputs stream per batch (x in 2 chunks, then h), alternating, so
    batch-0 transposes start ~3us in; gram rows i need only x tiles
    0..i.  Gate fold, blockdiag aug matmul, ones-column denominators as
    in the baseline.  PSUM: 2 banks transposes + 4 banks d2 + 2 banks
    hW/PV = exactly 8.
"""

import sys

import numpy as np

if "/opt/trn_rl_repo" not in sys.path:
    sys.path.append("/opt/trn_rl_repo")

import concourse.bacc as bacc
import concourse.bass as bass
import concourse.mybir as mybir
import concourse.tile as tile
from concourse.bass import ts
from concourse.bass_utils import run_bass_kernel_spmd
from concourse.masks import make_identity

F32 = mybir.dt.float32
BF16 = mybir.dt.bfloat16
FP8 = mybir.dt.float8e4
DR = mybir.MatmulPerfMode.DoubleRow
AF = mybir.ActivationFunctionType
OP = mybir.AluOpType

S = 1024
B = 32
NCORES = 8
BS = B // NCORES  # batches per core
E = 256
H = 256
A = 256
NT = S // 128  # 8 t-tiles
MARGIN = 4.0  # replaces max(d2,0)+eps; absorbs rounding (cancels in softmax)

# lower-triangle compact layout: row-block i holds cols j in [0, 128*(i+1))
NI = [128 * (i + 1) for i in range(NT)]
OFF = [64 * i * (i + 1) for i in range(NT)]  # sum of NI[:i]
TRI = OFF[NT - 1] + NI[NT - 1]  # 4608
LOFF = [64 * i * (i - 1) for i in range(NT)]  # p_lo group offset: 128*(0+1+..+i-1)
NLO = 28 * 128  # 3584


def _chain(prev, cur, reason):
    """Pin same-engine execution order (table-set discipline)."""
    if prev is not None:
        tile.add_dep_helper(cur.ins, prev.ins, sync=False, reason=reason)
    return cur


def build_graph():
    nc = bacc.Bacc("TRN2", target_bir_lowering=False, debug=False)

    x_ext = nc.declare_dram_parameter("x", [S, BS, E], F32, isOutput=False)
    h_ext = nc.declare_dram_parameter("h", [S, BS, H], F32, isOutput=False)
    w_ext = nc.declare_dram_parameter("w_sim", [1, 1], F32, isOutput=False)
    wg_ext = nc.declare_dram_parameter("W_g", [A, 2 * H], F32, isOutput=False)
    bg_ext = nc.declare_dram_parameter("b_g", [1, A], F32, isOutput=False)
    out_ext = nc.declare_dram_parameter("out", [S, BS, A], F32, isOutput=True)

    with tile.TileContext(nc) as tc:
        with (
            tc.tile_pool(name="consts", bufs=1) as consts,
            tc.tile_pool(name="nat", bufs=2) as natp,
            tc.tile_pool(name="xt", bufs=2) as xtp,
            tc.tile_pool(name="small", bufs=2) as smallp,
            tc.tile_pool(name="dcmp", bufs=BS) as dcmpp,
            tc.tile_pool(name="pcmp", bufs=2) as pcmpp,
            tc.tile_pool(name="plo", bufs=2) as plop,
            tc.tile_pool(name="hw", bufs=BS) as hwp,
            tc.tile_pool(name="zt", bufs=2) as ztp,
            tc.tile_pool(name="ps_tr", bufs=2, space="PSUM") as pstr,
        ):
            big_ctx = tc.tile_pool(name="ps_big", bufs=2, space="PSUM")
            psbig = big_ctx.__enter__()
            # ---- PE HAM warm-up: dense burst while input DMAs fly ----
            warm_in = consts.tile([128, 128], BF16)
            nc.vector.memset(warm_in, 1.0)
            warm_ps = psbig.tile([128, 512], F32, tag="big")
            for _ in range(72):
                nc.tensor.matmul(
                    warm_ps[:, 0:128], warm_in[:], warm_in[:], start=True, stop=True
                )

            # ---- input DMAs: stream per batch, x (2 chunks) then h ----
            xnat_list, hnat_list = [], []
            for b in range(BS):
                xnat = natp.tile([128, NT, E], BF16, tag="xnat")
                xnat_list.append(xnat)
                xsrc = x_ext[:, b, :].rearrange("(i p) e -> p i e", p=128)
                cw = 4
                for g in range(NT // cw):
                    nc.gpsimd.dma_start(
                        out=xnat[:, cw * g : cw * (g + 1), :],
                        in_=xsrc[:, cw * g : cw * (g + 1), :],
                    )
                hnat = natp.tile([128, NT, H], BF16, tag="hnat")
                hnat_list.append(hnat)
                nc.gpsimd.dma_start(
                    out=hnat,
                    in_=h_ext[:, b, :].rearrange("(i p) e -> p i e", p=128),
                )

            # ---------------- constants ----------------
            ident = consts.tile([128, 128], F32)
            make_identity(nc, ident)
            identb = consts.tile([128, 128], BF16)
            nc.vector.tensor_copy(identb, ident)
            ones_stage = consts.tile([8, 128], F32)
            nc.vector.memset(ones_stage, 1.0)
            ones_row = consts.tile([1, 128], BF16)
            nc.vector.tensor_copy(ones_row, ones_stage[0:1, :])
            ones8 = consts.tile([8, 128], BF16)
            nc.vector.tensor_copy(ones8, ones_stage)

            w_col = consts.tile([128, 1], F32)
            nc.sync.dma_start(out=w_col, in_=w_ext[:].partition_broadcast(128))
            bg_stage = consts.tile([1, A], F32)
            nc.sync.dma_start(out=bg_stage, in_=bg_ext[:])
            bg_row = consts.tile([1, A], BF16)
            nc.vector.tensor_copy(bg_row, bg_stage)

            # W_g (A, 2H) -> W12T: 2 k-tiles of (128hd, [A | A]) used as hW rhs
            wnat = consts.tile([128, 2, 2 * H], F32)
            nc.sync.dma_start(
                out=wnat, in_=wg_ext[:].rearrange("(m p) k -> p m k", m=2)
            )
            w12t = consts.tile([128, 2, 2 * H], BF16)
            for k2 in range(2):
                ps = psbig.tile([128, 512], F32, tag="big")
                for w in range(2):
                    for m in range(2):
                        nc.tensor.transpose(
                            ps[:, w * 256 + m * 128 : w * 256 + (m + 1) * 128],
                            wnat[:, m, w * 256 + k2 * 128 : w * 256 + (k2 + 1) * 128],
                            ident[:],
                        )
                nc.vector.tensor_copy(w12t[:, k2, :], ps[:])

            # force the Sqrt table set to load once, up front
            dummy = consts.tile([128, 1], F32)
            sc_prev = nc.scalar.activation(out=dummy, in_=w_col, func=AF.Sqrt)

            # -------- per-batch state --------
            d_cmp = [dcmpp.tile([128, TRI], BF16, tag="d", name=f"dcmp{b}") for b in range(BS)]
            hw_l = [hwp.tile([128, NT, 520], BF16, tag="hw", name=f"hw{b}") for b in range(BS)]

            def transpose_nat(nat, dstT):
                """nat [128, NT, 256] -> dstT [128, 2, S] (k-major)."""
                for g in range(2):
                    psT = pstr.tile([128, 1024], BF16, tag="tr")
                    for i2 in range(4):
                        i = 4 * g + i2
                        for k2 in range(2):
                            nc.tensor.transpose(
                                psT[:, i2 * 256 + k2 * 128 : i2 * 256 + (k2 + 1) * 128],
                                nat[:, i, ts(k2, 128)],
                                identb[:],
                            )
                    nc.vector.tensor_copy(
                        dstT[:, :, 512 * g : 512 * (g + 1)].rearrange(
                            "p k (i f) -> p i k f", i=4
                        ),
                        psT[:].rearrange("p (i k f) -> p i k f", i=4, k=2),
                    )

            def do_hw(b):
                """hW = h @ [W1|W2]^T (+ bg on the W2 half) + ones column."""
                hT = xtp.tile([128, 2, S], BF16, tag="hT")
                transpose_nat(hnat_list[b], hT)
                hw = hw_l[b]
                for m in range(NT):
                    ps = psbig.tile([128, 512], F32, tag="big")
                    nc.tensor.matmul(
                        ps[:], hT[:, 0, ts(m, 128)], w12t[:, 0, :],
                        start=True, stop=False,
                    )
                    nc.tensor.matmul(
                        ps[:], hT[:, 1, ts(m, 128)], w12t[:, 1, :],
                        start=False, stop=False,
                    )
                    nc.tensor.matmul(
                        ps[:, 256:512], ones_row[:], bg_row[:],
                        start=False, stop=True,
                    )
                    hwm = hw[:, m, :]
                    dst = bass.AP(
                        tensor=hwm.tensor,
                        offset=hwm.offset,
                        ap=[hwm.ap[0], [257, 2], [1, 256]],
                    )
                    nc.vector.tensor_copy(dst, ps[:].rearrange("p (u f) -> p u f", u=2))
                nc.vector.memset(hw[:, :, 256:257], 1.0)

            # ================= stage 1: x->dist triangle (+ hW) =================
            stage1 = tc.tile_pool(name="ps_d2", bufs=2, space="PSUM")
            psd2 = stage1.__enter__()
            for b in range(BS):
                xnat = xnat_list[b]
                xT = xtp.tile([128, 2, S], BF16, tag="xT")
                transpose_nat(xnat, xT)

                # per x-chunk g: |x_t|^2 (DVE) -> bias half + blockdiag half
                bias_h = []
                bd_h = []
                for g in range(2):
                    sqm = smallp.tile([128, 4], F32, tag=f"sqm{g}", name=f"sqm{g}")
                    for u in range(4):
                        i = 4 * g + u
                        scr = smallp.tile([128, E], BF16, tag="scr")
                        nc.vector.scalar_tensor_tensor(
                            out=scr,
                            in0=xnat[:, i, :],
                            scalar=1.0,
                            in1=xnat[:, i, :],
                            op0=OP.mult,
                            op1=OP.mult,
                            accum_out=sqm[:, u : u + 1],
                        )
                    biasg = smallp.tile([128, 4], F32, tag=f"bias{g}", name=f"bias{g}")
                    bias_h.append(biasg)
                    nc.vector.tensor_scalar_add(out=biasg, in0=sqm, scalar1=MARGIN)
                    sqmb = smallp.tile([128, 4], BF16, tag=f"sqmb{g}", name=f"sqmb{g}")
                    nc.vector.tensor_copy(sqmb[:], sqm[:])
                    sq4 = pstr.tile([4, 128], BF16, tag="tr", name="sq4")
                    nc.tensor.transpose(sq4[:], sqmb[:], identb[:])
                    sq4sb = smallp.tile([4, 128], BF16, tag=f"sq4sb{g}", name=f"sq4sb{g}")
                    nc.vector.tensor_scalar_mul(sq4sb[:], sq4[:], -0.5)
                    bdg = smallp.tile([8, 512], BF16, tag=f"bd{g}", name=f"bd{g}")
                    bd_h.append(bdg)
                    nc.vector.memset(bdg, 0.0)
                    bd = bdg[:]
                    diag_view = bass.AP(
                        tensor=bd.tensor, offset=bd.offset, ap=[[512 + 128, 4], [1, 128]]
                    )
                    nc.sync.dma_start(out=diag_view, in_=sq4sb[:])

                # gram rows (lower triangle) -> d2 psum -> sqrt -> compact dist
                for i in range(NT):
                    ni = NI[i]
                    d2 = psd2.tile([128, 1024], F32, tag="d2")
                    nchunk = (ni + 511) // 512
                    for c in range(nchunk):
                        j0 = 512 * c
                        j1 = min(ni, j0 + 512)
                        for k in range(2):
                            nc.tensor.matmul(
                                d2[:, j0:j1],
                                xT[:, k, ts(i, 128)],
                                xT[:, k, j0:j1],
                                start=(k == 0),
                                stop=False,
                            )
                        nc.tensor.matmul(
                            d2[:, j0:j1],
                            ones8[:],
                            bd_h[c][:, 0 : j1 - j0],
                            start=False,
                            stop=True,
                        )
                    si = nc.scalar.activation(
                        out=d_cmp[b][:, OFF[i] : OFF[i] + ni],
                        in_=d2[:, 0:ni],
                        func=AF.Sqrt,
                        bias=bias_h[i // 4][:, i % 4 : i % 4 + 1],
                        scale=-2.0,
                    )
                    sc_prev = _chain(sc_prev, si, "act-order")

                if b < 2:
                    do_hw(b)
            do_hw(2)
            do_hw(3)
            stage1.__exit__(None, None, None)
            big_ctx.__exit__(None, None, None)
            stage2 = tc.tile_pool(name="ps_pv", bufs=6, space="PSUM")
            pspv = stage2.__enter__()

            # ================= stage 2: exp, PV, gate =================
            sc_box = [sc_prev]
            p_cmp_l = {}

            def emit_exp(b):
                if b in p_cmp_l:
                    return
                pc = pcmpp.tile([128, TRI], BF16, tag="p", name=f"pcmp{b}")
                ei = nc.scalar.activation(
                    out=pc, in_=d_cmp[b], func=AF.Exp, scale=w_col[:, 0:1]
                )
                sc_box[0] = _chain(sc_box[0], ei, "act-order")
                p_cmp_l[b] = pc

            emit_exp(0)
            for b in range(BS):
                p_cmp = p_cmp_l[b]
                p_lo = plop.tile([128, NLO], BF16, tag="plo", name=f"plo{b}")
                hw = hw_l[b]

                def emit_transp(i):
                    # transpose the missing lhsT tiles (k < i) for PV_i
                    psT = pstr.tile([128, 1024], BF16, tag="tr")
                    for k in range(i):
                        nc.tensor.transpose(
                            psT[:, ts(k, 128)],
                            p_cmp[:, OFF[i] + 128 * k : OFF[i] + 128 * (k + 1)],
                            identb[:],
                        )
                    dst = p_lo[:, LOFF[i] : LOFF[i] + 128 * i]
                    if i <= 3:
                        ci = nc.scalar.copy(dst, psT[:, 0 : 128 * i])
                        sc_box[0] = _chain(sc_box[0], ci, "act-order")
                    else:
                        nc.vector.tensor_copy(dst, psT[:, 0 : 128 * i])

                zs = None
                for i in range(NT):
                    if i + 1 < NT:
                        emit_transp(i + 1)
                    if i == 1 and b + 1 < BS:
                        emit_exp(b + 1)
                    if i % 4 == 0:
                        zs = ztp.tile([128, 4, A], F32, tag="zs", bufs=3)
                    pv = pspv.tile([128, 512], F32, tag="pv")
                    for k in range(NT):
                        if k >= i:
                            lhsT = p_cmp[:, OFF[k] + 128 * i : OFF[k] + 128 * (i + 1)]
                        else:
                            lhsT = p_lo[:, LOFF[i] + 128 * k : LOFF[i] + 128 * (k + 1)]
                        nc.tensor.matmul(
                            pv[:, 0 : A + 1],
                            lhsT,
                            hw[:, k, 0 : A + 1],
                            start=(k == 0),
                            stop=(k == NT - 1),
                        )
                    rp_i = smallp.tile([128, 1], F32, tag="rp_i")
                    nc.vector.reciprocal(rp_i[:], pv[:, A : A + 1])
                    nc.vector.scalar_tensor_tensor(
                        out=zs[:, i % 4, :],
                        in0=pv[:, 0:A],
                        scalar=rp_i[:, 0:1],
                        in1=hw[:, i, 257 : 257 + A],
                        op0=OP.mult,
                        op1=OP.add,
                    )
                    if i % 4 == 3:
                        g2 = i // 4
                        zo = ztp.tile([128, 4, A], F32, tag="zo")
                        ti = nc.scalar.activation(
                            out=zo[:].rearrange("p a b -> p (a b)"),
                            in_=zs[:].rearrange("p a b -> p (a b)"),
                            func=AF.Tanh,
                        )
                        sc_box[0] = _chain(sc_box[0], ti, "act-order")
                        for q in range(2):
                            r0 = 512 * g2 + 256 * q
                            nc.gpsimd.dma_start(
                                out=out_ext[r0 : r0 + 256, b, :].rearrange(
                                    "(u p) a -> p u a", p=128
                                ),
                                in_=zo[:, 2 * q : 2 * q + 2, :],
                            )
            stage2.__exit__(None, None, None)

    nc.compile()
    return nc


_CACHED = {}


def _get_graph():
    if "nc" not in _CACHED:
        _CACHED["nc"] = build_graph()
    return _CACHED["nc"]


def _run(inputs, trace=False, **kw):
    nc = _get_graph()
    x = np.asarray(inputs["x"], dtype=np.float32)
    h = np.asarray(inputs["h"], dtype=np.float32)
    w_sim = np.asarray(inputs["w_sim"], dtype=np.float32).reshape(1, 1)
    W_g = np.ascontiguousarray(np.asarray(inputs["W_g"], dtype=np.float32))
    b_g = np.asarray(inputs["b_g"], dtype=np.float32).reshape(1, A)
    in_maps = []
    for c in range(NCORES):
        in_maps.append(
            {
                "x": np.ascontiguousarray(x[:, c * BS : (c + 1) * BS, :]),
                "h": np.ascontiguousarray(h[:, c * BS : (c + 1) * BS, :]),
                "w_sim": w_sim,
                "W_g": W_g,
                "b_g": b_g,
            }
        )
    res = run_bass_kernel_spmd(nc, in_maps, list(range(NCORES)), trace=trace, **kw)
    out = np.concatenate([res.results[c]["out"] for c in range(NCORES)], axis=1)
    return out, res


def kernel(**inputs):
    out, _ = _run(inputs, trace=False)
    return out


if __name__ == "__main__":
    rng = np.random.default_rng(0)
    ins = {
        "x": rng.standard_normal((S, B, E), dtype=np.float32),
        "h": rng.standard_normal((S, B, H), dtype=np.float32),
        "w_sim": np.array([0.03], dtype=np.float32),
        "b_sim": np.array([0.01], dtype=np.float32),
        "W_g": (rng.standard_normal((A, 2 * H)) * 0.05).astype(np.float32),
        "b_g": np.zeros(A, dtype=np.float32),
    }
    out = kernel(**ins)
    print("out", out.shape, out.dtype, np.abs(out).mean())


# revision 12
# speedup vs baseline: 1.2474x; 1.0434x over previous
"""Distance-based attention (nn_Attention_67989332296336) on 8 TRN2 NeuronCores.

Math per batch element b (S=1024, E=H=A=256):
    d2[t,j]  = |x_t|^2 + |x_j|^2 - 2 x_t.x_j
    dist     = sqrt(max(d2,0)+eps)
    scores   = w_sim*dist + b_sim
    A        = softmax_j(scores)
    G        = A @ h
    Z        = tanh([G, h] @ W_g^T + b_g)

Sharding: batch dim B=32 split over 8 cores (4 per core), weights replicated.

v2 strategy (vs the phase-split baseline):
  - Symmetry: dist/P are symmetric per batch, so the gram, sqrt and exp
    passes run only on the lower block-triangle (36/64 tiles), stored
    compacted ([128, 4608] per batch).  PV lhsT tiles for k >= i come
    straight out of the compact store; the 28 missing (k < i) tiles are
    128x128 PE transposes of stored tiles (bf16, FWL) copied out of PSUM
    by ScalarE (small groups) / DVE (large groups).
  - |x_t|^2 squares run on ScalarE (Square activation + accum_out),
    freeing DVE; sqrt applies scale=-2 / bias=|x_t|^2+MARGIN per i-tile;
    exp is one [128,4608] instruction per batch (scale=w_sim).
  - ScalarE runs exactly two table sets: all Square/Sqrt (stage 1),
    then all Exp/Copy/Tanh (stage 2), pinned by same-engine deps.
  - Inputs stream per batch (x in 2 chunks, then h), alternating, so
    batch-0 transposes start ~3us in; gram rows i need only x tiles
    0..i.  Gate fold, blockdiag aug matmul, ones-column denominators as
    in the baseline.  PSUM: 2 banks transposes + 4 banks d2 + 2 banks
    hW/PV = exactly 8.
"""

import sys

import numpy as np

if "/opt/trn_rl_repo" not in sys.path:
    sys.path.append("/opt/trn_rl_repo")

import concourse.bacc as bacc
import concourse.bass as bass
import concourse.mybir as mybir
import concourse.tile as tile
from concourse.bass import ts
from concourse.bass_utils import run_bass_kernel_spmd
from concourse.masks import make_identity

F32 = mybir.dt.float32
BF16 = mybir.dt.bfloat16
FP8 = mybir.dt.float8e4
DR = mybir.MatmulPerfMode.DoubleRow
AF = mybir.ActivationFunctionType
OP = mybir.AluOpType

S = 1024
B = 32
NCORES = 8
BS = B // NCORES  # batches per core
E = 256
H = 256
A = 256
NT = S // 128  # 8 t-tiles
MARGIN = 4.0  # replaces max(d2,0)+eps; absorbs rounding (cancels in softmax)

# lower-triangle compact layout: row-block i holds cols j in [0, 128*(i+1))
NI = [128 * (i + 1) for i in range(NT)]
OFF = [64 * i * (i + 1) for i in range(NT)]  # sum of NI[:i]
TRI = OFF[NT - 1] + NI[NT - 1]  # 4608
LOFF = [64 * i * (i - 1) for i in range(NT)]  # p_lo group offset: 128*(0+1+..+i-1)
NLO = 28 * 128  # 3584


def _chain(prev, cur, reason):
    """Pin same-engine execution order (table-set discipline)."""
    if prev is not None:
        tile.add_dep_helper(cur.ins, prev.ins, sync=False, reason=reason)
    return cur


def build_graph():
    nc = bacc.Bacc("TRN2", target_bir_lowering=False, debug=False)

    x_ext = nc.declare_dram_parameter("x", [S, BS, E], F32, isOutput=False)
    h_ext = nc.declare_dram_parameter("h", [S, BS, H], F32, isOutput=False)
    w_ext = nc.declare_dram_parameter("w_sim", [1, 1], F32, isOutput=False)
    wg_ext = nc.declare_dram_parameter("W_g", [A, 2 * H], F32, isOutput=False)
    bg_ext = nc.declare_dram_parameter("b_g", [1, A], F32, isOutput=False)
    out_ext = nc.declare_dram_parameter("out", [S, BS, A], F32, isOutput=True)

    with tile.TileContext(nc) as tc:
        with (
            tc.tile_pool(name="consts", bufs=1) as consts,
            tc.tile_pool(name="nat", bufs=2) as natp,
            tc.tile_pool(name="xt", bufs=2) as xtp,
            tc.tile_pool(name="small", bufs=2) as smallp,
            tc.tile_pool(name="dcmp", bufs=BS) as dcmpp,
            tc.tile_pool(name="pcmp", bufs=2) as pcmpp,
            tc.tile_pool(name="plo", bufs=2) as plop,
            tc.tile_pool(name="hw", bufs=BS) as hwp,
            tc.tile_pool(name="zt", bufs=2) as ztp,
            tc.tile_pool(name="ps_tr", bufs=2, space="PSUM") as pstr,
        ):
            big_ctx = tc.tile_pool(name="ps_big", bufs=2, space="PSUM")
            psbig = big_ctx.__enter__()
            # ---- PE HAM warm-up: dense burst while input DMAs fly ----
            warm_in = consts.tile([128, 128], BF16)
            nc.vector.memset(warm_in, 1.0)
            warm_ps = psbig.tile([128, 512], F32, tag="big")
            for _ in range(72):
                nc.tensor.matmul(
                    warm_ps[:, 0:128], warm_in[:], warm_in[:], start=True, stop=True
                )

            # ---- input DMAs: stream per batch, x (2 chunks) then h ----
            xnat_list, hnat_list = [], []
            for b in range(BS):
                xnat = natp.tile([128, NT, E], BF16, tag="xnat")
                xnat_list.append(xnat)
                xsrc = x_ext[:, b, :].rearrange("(i p) e -> p i e", p=128)
                cw = 4
                for g in range(NT // cw):
                    nc.gpsimd.dma_start(
                        out=xnat[:, cw * g : cw * (g + 1), :],
                        in_=xsrc[:, cw * g : cw * (g + 1), :],
                    )
                hnat = natp.tile([128, NT, H], BF16, tag="hnat")
                hnat_list.append(hnat)
                nc.gpsimd.dma_start(
                    out=hnat,
                    in_=h_ext[:, b, :].rearrange("(i p) e -> p i e", p=128),
                )

            # ---------------- constants ----------------
            ident = consts.tile([128, 128], F32)
            make_identity(nc, ident)
            identb = consts.tile([128, 128], BF16)
            nc.vector.tensor_copy(identb, ident)
            ones_stage = consts.tile([8, 128], F32)
            nc.vector.memset(ones_stage, 1.0)
            ones_row = consts.tile([1, 128], BF16)
            nc.vector.tensor_copy(ones_row, ones_stage[0:1, :])
            ones8 = consts.tile([8, 128], BF16)
            nc.vector.tensor_copy(ones8, ones_stage)

            w_col = consts.tile([128, 1], F32)
            nc.sync.dma_start(out=w_col, in_=w_ext[:].partition_broadcast(128))
            bg_stage = consts.tile([1, A], F32)
            nc.sync.dma_start(out=bg_stage, in_=bg_ext[:])
            bg_row = consts.tile([1, A], BF16)
            nc.vector.tensor_copy(bg_row, bg_stage)

            # W_g (A, 2H) -> W12T: 2 k-tiles of (128hd, [A | A]) used as hW rhs
            wnat = consts.tile([128, 2, 2 * H], F32)
            nc.sync.dma_start(
                out=wnat, in_=wg_ext[:].rearrange("(m p) k -> p m k", m=2)
            )
            w12t = consts.tile([128, 2, 2 * H], BF16)
            for k2 in range(2):
                ps = psbig.tile([128, 512], F32, tag="big")
                for w in range(2):
                    for m in range(2):
                        nc.tensor.transpose(
                            ps[:, w * 256 + m * 128 : w * 256 + (m + 1) * 128],
                            wnat[:, m, w * 256 + k2 * 128 : w * 256 + (k2 + 1) * 128],
                            ident[:],
                        )
                nc.vector.tensor_copy(w12t[:, k2, :], ps[:])

            # force the Sqrt table set to load once, up front
            dummy = consts.tile([128, 1], F32)
            sc_prev = nc.scalar.activation(out=dummy, in_=w_col, func=AF.Sqrt)

            # -------- per-batch state --------
            d_cmp = [dcmpp.tile([128, TRI], BF16, tag="d", name=f"dcmp{b}") for b in range(BS)]
            hw_l = [hwp.tile([128, NT, 520], BF16, tag="hw", name=f"hw{b}") for b in range(BS)]

            def transpose_nat(nat, dstT):
                """nat [128, NT, 256] -> dstT [128, 2, S] (k-major)."""
                for g in range(2):
                    psT = pstr.tile([128, 1024], BF16, tag="tr")
                    for i2 in range(4):
                        i = 4 * g + i2
                        for k2 in range(2):
                            nc.tensor.transpose(
                                psT[:, i2 * 256 + k2 * 128 : i2 * 256 + (k2 + 1) * 128],
                                nat[:, i, ts(k2, 128)],
                                identb[:],
                            )
                    nc.vector.tensor_copy(
                        dstT[:, :, 512 * g : 512 * (g + 1)].rearrange(
                            "p k (i f) -> p i k f", i=4
                        ),
                        psT[:].rearrange("p (i k f) -> p i k f", i=4, k=2),
                    )

            def do_hw(b):
                """hW = h @ [W1|W2]^T (+ bg on the W2 half) + ones column."""
                hT = xtp.tile([128, 2, S], BF16, tag="hT")
                transpose_nat(hnat_list[b], hT)
                hw = hw_l[b]
                for m in range(NT):
                    ps = psbig.tile([128, 512], F32, tag="big")
                    nc.tensor.matmul(
                        ps[:], hT[:, 0, ts(m, 128)], w12t[:, 0, :],
                        start=True, stop=False,
                    )
                    nc.tensor.matmul(
                        ps[:], hT[:, 1, ts(m, 128)], w12t[:, 1, :],
                        start=False, stop=False,
                    )
                    nc.tensor.matmul(
                        ps[:, 256:512], ones_row[:], bg_row[:],
                        start=False, stop=True,
                    )
                    hwm = hw[:, m, :]
                    dst = bass.AP(
                        tensor=hwm.tensor,
                        offset=hwm.offset,
                        ap=[hwm.ap[0], [257, 2], [1, 256]],
                    )
                    nc.vector.tensor_copy(dst, ps[:].rearrange("p (u f) -> p u f", u=2))
                nc.vector.memset(hw[:, :, 256:257], 1.0)

            # ================= stage 1: x->dist triangle (+ hW) =================
            stage1 = tc.tile_pool(name="ps_d2", bufs=2, space="PSUM")
            psd2 = stage1.__enter__()
            for b in range(BS):
                xnat = xnat_list[b]
                xT = xtp.tile([128, 2, S], BF16, tag="xT")
                transpose_nat(xnat, xT)

                # per x-chunk g: |x_t|^2 (DVE) -> bias half + blockdiag half
                bias_h = []
                bd_h = []
                for g in range(2):
                    sqm = smallp.tile([128, 4], F32, tag=f"sqm{g}", name=f"sqm{g}")
                    for u in range(4):
                        i = 4 * g + u
                        scr = smallp.tile([128, E], BF16, tag="scr")
                        nc.vector.scalar_tensor_tensor(
                            out=scr,
                            in0=xnat[:, i, :],
                            scalar=1.0,
                            in1=xnat[:, i, :],
                            op0=OP.mult,
                            op1=OP.mult,
                            accum_out=sqm[:, u : u + 1],
                        )
                    biasg = smallp.tile([128, 4], F32, tag=f"bias{g}", name=f"bias{g}")
                    bias_h.append(biasg)
                    nc.vector.tensor_scalar_add(out=biasg, in0=sqm, scalar1=MARGIN)
                    sqmb = smallp.tile([128, 4], BF16, tag=f"sqmb{g}", name=f"sqmb{g}")
                    nc.vector.tensor_copy(sqmb[:], sqm[:])
                    sq4 = pstr.tile([4, 128], BF16, tag="tr", name="sq4")
                    nc.tensor.transpose(sq4[:], sqmb[:], identb[:])
                    sq4sb = smallp.tile([4, 128], BF16, tag=f"sq4sb{g}", name=f"sq4sb{g}")
                    nc.vector.tensor_scalar_mul(sq4sb[:], sq4[:], -0.5)
                    bdg = smallp.tile([8, 512], BF16, tag=f"bd{g}", name=f"bd{g}")
                    bd_h.append(bdg)
                    nc.vector.memset(bdg, 0.0)
                    bd = bdg[:]
                    diag_view = bass.AP(
                        tensor=bd.tensor, offset=bd.offset, ap=[[512 + 128, 4], [1, 128]]
                    )
                    nc.sync.dma_start(out=diag_view, in_=sq4sb[:])

                # gram rows (lower triangle) -> d2 psum -> sqrt -> compact dist
                for i in range(NT):
                    ni = NI[i]
                    d2 = psd2.tile([128, 1024], F32, tag="d2")
                    nchunk = (ni + 511) // 512
                    for c in range(nchunk):
                        j0 = 512 * c
                        j1 = min(ni, j0 + 512)
                        for k in range(2):
                            nc.tensor.matmul(
                                d2[:, j0:j1],
                                xT[:, k, ts(i, 128)],
                                xT[:, k, j0:j1],
                                start=(k == 0),
                                stop=False,
                            )
                        nc.tensor.matmul(
                            d2[:, j0:j1],
                            ones8[:],
                            bd_h[c][:, 0 : j1 - j0],
                            start=False,
                            stop=True,
                        )
                    si = nc.scalar.activation(
                        out=d_cmp[b][:, OFF[i] : OFF[i] + ni],
                        in_=d2[:, 0:ni],
                        func=AF.Sqrt,
                        bias=bias_h[i // 4][:, i % 4 : i % 4 + 1],
                        scale=-2.0,
                    )
                    sc_prev = _chain(sc_prev, si, "act-order")

                if b < 2:
                    do_hw(b)
            do_hw(2)
            do_hw(3)
            stage1.__exit__(None, None, None)
            big_ctx.__exit__(None, None, None)
            stage2 = tc.tile_pool(name="ps_pv", bufs=6, space="PSUM")
            pspv = stage2.__enter__()

            # ================= stage 2: exp, PV, gate =================
            sc_box = [sc_prev]
            p_cmp_l = {}

            def emit_exp(b):
                if b in p_cmp_l:
                    return
                pc = pcmpp.tile([128, TRI], BF16, tag="p", name=f"pcmp{b}")
                ei = nc.scalar.activation(
                    out=pc, in_=d_cmp[b], func=AF.Exp, scale=w_col[:, 0:1]
                )
                sc_box[0] = _chain(sc_box[0], ei, "act-order")
                p_cmp_l[b] = pc

            emit_exp(0)
            for b in range(BS):
                p_cmp = p_cmp_l[b]
                p_lo = plop.tile([128, NLO], BF16, tag="plo", name=f"plo{b}")
                hw = hw_l[b]

                def emit_transp(i):
                    # transpose the missing lhsT tiles (k < i) for PV_i
                    psT = pstr.tile([128, 1024], BF16, tag="tr")
                    for k in range(i):
                        nc.tensor.transpose(
                            psT[:, ts(k, 128)],
                            p_cmp[:, OFF[i] + 128 * k : OFF[i] + 128 * (k + 1)],
                            identb[:],
                        )
                    dst = p_lo[:, LOFF[i] : LOFF[i] + 128 * i]
                    if i <= 3:
                        ci = nc.scalar.copy(dst, psT[:, 0 : 128 * i])
                        sc_box[0] = _chain(sc_box[0], ci, "act-order")
                    else:
                        nc.vector.tensor_copy(dst, psT[:, 0 : 128 * i])

                zs = None
                for i in range(NT):
                    if i + 1 < NT:
                        emit_transp(i + 1)
                    if i == 3 and b + 1 < BS:
                        emit_exp(b + 1)
                    if i % 4 == 0:
                        zs = ztp.tile([128, 4, A], F32, tag="zs", bufs=3)
                    pv = pspv.tile([128, 512], F32, tag="pv")
                    for k in range(NT):
                        if k >= i:
                            lhsT = p_cmp[:, OFF[k] + 128 * i : OFF[k] + 128 * (i + 1)]
                        else:
                            lhsT = p_lo[:, LOFF[i] + 128 * k : LOFF[i] + 128 * (k + 1)]
                        nc.tensor.matmul(
                            pv[:, 0 : A + 1],
                            lhsT,
                            hw[:, k, 0 : A + 1],
                            start=(k == 0),
                            stop=(k == NT - 1),
                        )
                    rp_i = smallp.tile([128, 1], F32, tag="rp_i")
                    nc.vector.reciprocal(rp_i[:], pv[:, A : A + 1])
                    nc.vector.scalar_tensor_tensor(
                        out=zs[:, i % 4, :],
                        in0=pv[:, 0:A],
                        scalar=rp_i[:, 0:1],
                        in1=hw[:, i, 257 : 257 + A],
                        op0=OP.mult,
                        op1=OP.add,
                    )
                    if i % 4 == 3:
                        g2 = i // 4
                        zo = ztp.tile([128, 4, A], F32, tag="zo")
                        ti = nc.scalar.activation(
                            out=zo[:].rearrange("p a b -> p (a b)"),
                            in_=zs[:].rearrange("p a b -> p (a b)"),
                            func=AF.Tanh,
                        )
                        sc_box[0] = _chain(sc_box[0], ti, "act-order")
                        for q in range(2):
                            r0 = 512 * g2 + 256 * q
                            nc.gpsimd.dma_start(
                                out=out_ext[r0 : r0 + 256, b, :].rearrange(
                                    "(u p) a -> p u a", p=128
                                ),
                                in_=zo[:, 2 * q : 2 * q + 2, :],
                            )
            stage2.__exit__(None, None, None)

    nc.compile()
    return nc


_CACHED = {}


def _get_graph():
    if "nc" not in _CACHED:
        _CACHED["nc"] = build_graph()
    return _CACHED["nc"]


def _run(inputs, trace=False, **kw):
    nc = _get_graph()
    x = np.asarray(inputs["x"], dtype=np.float32)
    h = np.asarray(inputs["h"], dtype=np.float32)
    w_sim = np.asarray(inputs["w_sim"], dtype=np.float32).reshape(1, 1)
    W_g = np.ascontiguousarray(np.asarray(inputs["W_g"], dtype=np.float32))
    b_g = np.asarray(inputs["b_g"], dtype=np.float32).reshape(1, A)
    in_maps = []
    for c in range(NCORES):
        in_maps.append(
            {
                "x": np.ascontiguousarray(x[:, c * BS : (c + 1) * BS, :]),
                "h": np.ascontiguousarray(h[:, c * BS : (c + 1) * BS, :]),
                "w_sim": w_sim,
                "W_g": W_g,
                "b_g": b_g,
            }
        )
    res = run_bass_kernel_spmd(nc, in_maps, list(range(NCORES)), trace=trace, **kw)
    out = np.concatenate([res.results[c]["out"] for c in range(NCORES)], axis=1)
    return out, res


def kernel(**inputs):
    out, _ = _run(inputs, trace=False)
    return out


if __name__ == "__main__":
    rng = np.random.default_rng(0)
    ins = {
        "x": rng.standard_normal((S, B, E), dtype=np.float32),
        "h": rng.standard_normal((S, B, H), dtype=np.float32),
        "w_sim": np.array([0.03], dtype=np.float32),
        "b_sim": np.array([0.01], dtype=np.float32),
        "W_g": (rng.standard_normal((A, 2 * H)) * 0.05).astype(np.float32),
        "b_g": np.zeros(A, dtype=np.float32),
    }
    out = kernel(**ins)
    print("out", out.shape, out.dtype, np.abs(out).mean())


# revision 14
# speedup vs baseline: 1.2878x; 1.0324x over previous
"""Distance-based attention (nn_Attention_67989332296336) on 8 TRN2 NeuronCores.

Math per batch element b (S=1024, E=H=A=256):
    d2[t,j]  = |x_t|^2 + |x_j|^2 - 2 x_t.x_j
    dist     = sqrt(max(d2,0)+eps)
    scores   = w_sim*dist + b_sim
    A        = softmax_j(scores)
    G        = A @ h
    Z        = tanh([G, h] @ W_g^T + b_g)

Sharding: batch dim B=32 split over 8 cores (4 per core), weights replicated.

v2 strategy (vs the phase-split baseline):
  - Symmetry: dist/P are symmetric per batch, so the gram, sqrt and exp
    passes run only on the lower block-triangle (36/64 tiles), stored
    compacted ([128, 4608] per batch).  PV lhsT tiles for k >= i come
    straight out of the compact store; the 28 missing (k < i) tiles are
    128x128 PE transposes of stored tiles (bf16, FWL) copied out of PSUM
    by ScalarE (small groups) / DVE (large groups).
  - |x_t|^2 squares run on ScalarE (Square activation + accum_out),
    freeing DVE; sqrt applies scale=-2 / bias=|x_t|^2+MARGIN per i-tile;
    exp is one [128,4608] instruction per batch (scale=w_sim).
  - ScalarE runs exactly two table sets: all Square/Sqrt (stage 1),
    then all Exp/Copy/Tanh (stage 2), pinned by same-engine deps.
  - Inputs stream per batch (x in 2 chunks, then h), alternating, so
    batch-0 transposes start ~3us in; gram rows i need only x tiles
    0..i.  Gate fold, blockdiag aug matmul, ones-column denominators as
    in the baseline.  PSUM: 2 banks transposes + 4 banks d2 + 2 banks
    hW/PV = exactly 8.
"""

import sys

import numpy as np

if "/opt/trn_rl_repo" not in sys.path:
    sys.path.append("/opt/trn_rl_repo")

import concourse.bacc as bacc
import concourse.bass as bass
import concourse.mybir as mybir
import concourse.tile as tile
from concourse.bass import ts
from concourse.bass_utils import run_bass_kernel_spmd
from concourse.masks import make_identity

F32 = mybir.dt.float32
BF16 = mybir.dt.bfloat16
FP8 = mybir.dt.float8e4
DR = mybir.MatmulPerfMode.DoubleRow
AF = mybir.ActivationFunctionType
OP = mybir.AluOpType

S = 1024
B = 32
NCORES = 8
BS = B // NCORES  # batches per core
E = 256
H = 256
A = 256
NT = S // 128  # 8 t-tiles
MARGIN = 4.0  # replaces max(d2,0)+eps; absorbs rounding (cancels in softmax)

# lower-triangle compact layout: row-block i holds cols j in [0, 128*(i+1))
NI = [128 * (i + 1) for i in range(NT)]
OFF = [64 * i * (i + 1) for i in range(NT)]  # sum of NI[:i]
TRI = OFF[NT - 1] + NI[NT - 1]  # 4608
LOFF = [64 * i * (i - 1) for i in range(NT)]  # p_lo group offset: 128*(0+1+..+i-1)
NLO = 28 * 128  # 3584


def _chain(prev, cur, reason):
    """Pin same-engine execution order (table-set discipline)."""
    if prev is not None:
        tile.add_dep_helper(cur.ins, prev.ins, sync=False, reason=reason)
    return cur


def build_graph():
    nc = bacc.Bacc("TRN2", target_bir_lowering=False, debug=False)

    x_ext = nc.declare_dram_parameter("x", [S, BS, E], F32, isOutput=False)
    h_ext = nc.declare_dram_parameter("h", [S, BS, H], F32, isOutput=False)
    w_ext = nc.declare_dram_parameter("w_sim", [1, 1], F32, isOutput=False)
    wg_ext = nc.declare_dram_parameter("W_g", [A, 2 * H], F32, isOutput=False)
    bg_ext = nc.declare_dram_parameter("b_g", [1, A], F32, isOutput=False)
    out_ext = nc.declare_dram_parameter("out", [S, BS, A], F32, isOutput=True)

    with tile.TileContext(nc) as tc:
        with (
            tc.tile_pool(name="consts", bufs=1) as consts,
            tc.tile_pool(name="nat", bufs=2) as natp,
            tc.tile_pool(name="xt", bufs=2) as xtp,
            tc.tile_pool(name="small", bufs=2) as smallp,
            tc.tile_pool(name="dcmp", bufs=BS) as dcmpp,
            tc.tile_pool(name="pcmp", bufs=2) as pcmpp,
            tc.tile_pool(name="plo", bufs=2) as plop,
            tc.tile_pool(name="hw", bufs=BS) as hwp,
            tc.tile_pool(name="zt", bufs=2) as ztp,
            tc.tile_pool(name="ps_tr", bufs=2, space="PSUM") as pstr,
        ):
            big_ctx = tc.tile_pool(name="ps_big", bufs=2, space="PSUM")
            psbig = big_ctx.__enter__()
            # ---- PE HAM warm-up: dense burst while input DMAs fly ----
            warm_in = consts.tile([128, 128], BF16)
            nc.vector.memset(warm_in, 1.0)
            warm_ps = psbig.tile([128, 512], F32, tag="big")
            for _ in range(72):
                nc.tensor.matmul(
                    warm_ps[:, 0:128], warm_in[:], warm_in[:], start=True, stop=True
                )

            # ---- input DMAs: stream per batch, x (2 chunks) then h ----
            xnat_list, hnat_list = [], []
            for b in range(BS):
                xnat = natp.tile([128, NT, E], BF16, tag="xnat")
                xnat_list.append(xnat)
                xsrc = x_ext[:, b, :].rearrange("(i p) e -> p i e", p=128)
                cw = 4
                for g in range(NT // cw):
                    nc.gpsimd.dma_start(
                        out=xnat[:, cw * g : cw * (g + 1), :],
                        in_=xsrc[:, cw * g : cw * (g + 1), :],
                    )
                hnat = natp.tile([128, NT, H], BF16, tag="hnat")
                hnat_list.append(hnat)
                nc.gpsimd.dma_start(
                    out=hnat,
                    in_=h_ext[:, b, :].rearrange("(i p) e -> p i e", p=128),
                )

            # ---------------- constants ----------------
            ident = consts.tile([128, 128], F32)
            make_identity(nc, ident)
            identb = consts.tile([128, 128], BF16)
            nc.vector.tensor_copy(identb, ident)
            ones_stage = consts.tile([8, 128], F32)
            nc.vector.memset(ones_stage, 1.0)
            ones_row = consts.tile([1, 128], BF16)
            nc.vector.tensor_copy(ones_row, ones_stage[0:1, :])
            ones8 = consts.tile([8, 128], BF16)
            nc.vector.tensor_copy(ones8, ones_stage)

            w_col = consts.tile([128, 1], F32)
            nc.sync.dma_start(out=w_col, in_=w_ext[:].partition_broadcast(128))
            bg_stage = consts.tile([1, A], F32)
            nc.sync.dma_start(out=bg_stage, in_=bg_ext[:])
            bg_row = consts.tile([1, A], BF16)
            nc.vector.tensor_copy(bg_row, bg_stage)

            # W_g (A, 2H) -> W12T: 2 k-tiles of (128hd, [A | A]) used as hW rhs
            wnat = consts.tile([128, 2, 2 * H], F32)
            nc.sync.dma_start(
                out=wnat, in_=wg_ext[:].rearrange("(m p) k -> p m k", m=2)
            )
            w12t = consts.tile([128, 2, 2 * H], BF16)
            for k2 in range(2):
                ps = psbig.tile([128, 512], F32, tag="big")
                for w in range(2):
                    for m in range(2):
                        nc.tensor.transpose(
                            ps[:, w * 256 + m * 128 : w * 256 + (m + 1) * 128],
                            wnat[:, m, w * 256 + k2 * 128 : w * 256 + (k2 + 1) * 128],
                            ident[:],
                        )
                nc.vector.tensor_copy(w12t[:, k2, :], ps[:])

            # force the Sqrt table set to load once, up front
            dummy = consts.tile([128, 1], F32)
            sc_prev = nc.scalar.activation(out=dummy, in_=w_col, func=AF.Sqrt)

            # -------- per-batch state --------
            d_cmp = [dcmpp.tile([128, TRI], BF16, tag="d", name=f"dcmp{b}") for b in range(BS)]
            hw_l = [hwp.tile([128, NT, 520], BF16, tag="hw", name=f"hw{b}") for b in range(BS)]

            def transpose_nat(nat, dstT):
                """nat [128, NT, 256] -> dstT [128, 2, S] (k-major)."""
                for g in range(2):
                    psT = pstr.tile([128, 1024], BF16, tag="tr")
                    for i2 in range(4):
                        i = 4 * g + i2
                        for k2 in range(2):
                            nc.tensor.transpose(
                                psT[:, i2 * 256 + k2 * 128 : i2 * 256 + (k2 + 1) * 128],
                                nat[:, i, ts(k2, 128)],
                                identb[:],
                            )
                    nc.vector.tensor_copy(
                        dstT[:, :, 512 * g : 512 * (g + 1)].rearrange(
                            "p k (i f) -> p i k f", i=4
                        ),
                        psT[:].rearrange("p (i k f) -> p i k f", i=4, k=2),
                    )

            def do_hw(b):
                """hW = h @ [W1|W2]^T (+ bg on the W2 half) + ones column."""
                hT = xtp.tile([128, 2, S], BF16, tag="hT")
                transpose_nat(hnat_list[b], hT)
                hw = hw_l[b]
                for m in range(NT):
                    ps = psbig.tile([128, 512], F32, tag="big")
                    nc.tensor.matmul(
                        ps[:], hT[:, 0, ts(m, 128)], w12t[:, 0, :],
                        start=True, stop=False,
                    )
                    nc.tensor.matmul(
                        ps[:], hT[:, 1, ts(m, 128)], w12t[:, 1, :],
                        start=False, stop=False,
                    )
                    nc.tensor.matmul(
                        ps[:, 256:512], ones_row[:], bg_row[:],
                        start=False, stop=True,
                    )
                    hwm = hw[:, m, :]
                    dst = bass.AP(
                        tensor=hwm.tensor,
                        offset=hwm.offset,
                        ap=[hwm.ap[0], [257, 2], [1, 256]],
                    )
                    nc.vector.tensor_copy(dst, ps[:].rearrange("p (u f) -> p u f", u=2))
                nc.vector.memset(hw[:, :, 256:257], 1.0)

            # ================= stage 1: x->dist triangle (+ hW) =================
            stage1 = tc.tile_pool(name="ps_d2", bufs=2, space="PSUM")
            psd2 = stage1.__enter__()
            for b in range(BS):
                xnat = xnat_list[b]
                xT = xtp.tile([128, 2, S], BF16, tag="xT")
                transpose_nat(xnat, xT)

                # per x-chunk g: |x_t|^2 (DVE) -> bias half + blockdiag half
                bias_h = []
                bd_h = []
                for g in range(2):
                    sqm = smallp.tile([128, 4], F32, tag=f"sqm{g}", name=f"sqm{g}")
                    for u in range(4):
                        i = 4 * g + u
                        scr = smallp.tile([128, E], BF16, tag="scr")
                        nc.vector.scalar_tensor_tensor(
                            out=scr,
                            in0=xnat[:, i, :],
                            scalar=1.0,
                            in1=xnat[:, i, :],
                            op0=OP.mult,
                            op1=OP.mult,
                            accum_out=sqm[:, u : u + 1],
                        )
                    biasg = smallp.tile([128, 4], F32, tag=f"bias{g}", name=f"bias{g}")
                    bias_h.append(biasg)
                    nc.vector.tensor_scalar_add(out=biasg, in0=sqm, scalar1=MARGIN)
                    sqmb = smallp.tile([128, 4], BF16, tag=f"sqmb{g}", name=f"sqmb{g}")
                    nc.vector.tensor_copy(sqmb[:], sqm[:])
                    sq4 = pstr.tile([4, 128], BF16, tag="tr", name="sq4")
                    nc.tensor.transpose(sq4[:], sqmb[:], identb[:])
                    sq4sb = smallp.tile([4, 128], BF16, tag=f"sq4sb{g}", name=f"sq4sb{g}")
                    nc.vector.tensor_scalar_mul(sq4sb[:], sq4[:], -0.5)
                    bdg = smallp.tile([8, 512], BF16, tag=f"bd{g}", name=f"bd{g}")
                    bd_h.append(bdg)
                    nc.vector.memset(bdg, 0.0)
                    bd = bdg[:]
                    diag_view = bass.AP(
                        tensor=bd.tensor, offset=bd.offset, ap=[[512 + 128, 4], [1, 128]]
                    )
                    nc.sync.dma_start(out=diag_view, in_=sq4sb[:])

                # gram rows (lower triangle) -> d2 psum -> sqrt -> compact dist
                for i in range(NT):
                    ni = NI[i]
                    d2 = psd2.tile([128, 1024], F32, tag="d2")
                    nchunk = (ni + 511) // 512
                    for c in range(nchunk):
                        j0 = 512 * c
                        j1 = min(ni, j0 + 512)
                        for k in range(2):
                            nc.tensor.matmul(
                                d2[:, j0:j1],
                                xT[:, k, ts(i, 128)],
                                xT[:, k, j0:j1],
                                start=(k == 0),
                                stop=False,
                            )
                        nc.tensor.matmul(
                            d2[:, j0:j1],
                            ones8[:],
                            bd_h[c][:, 0 : j1 - j0],
                            start=False,
                            stop=True,
                        )
                    si = nc.scalar.activation(
                        out=d_cmp[b][:, OFF[i] : OFF[i] + ni],
                        in_=d2[:, 0:ni],
                        func=AF.Sqrt,
                        bias=bias_h[i // 4][:, i % 4 : i % 4 + 1],
                        scale=-2.0,
                    )
                    sc_prev = _chain(sc_prev, si, "act-order")

                if b < 2:
                    do_hw(b)
            do_hw(2)
            do_hw(3)
            stage1.__exit__(None, None, None)
            big_ctx.__exit__(None, None, None)
            stage2 = tc.tile_pool(name="ps_pv", bufs=6, space="PSUM")
            pspv = stage2.__enter__()

            # ================= stage 2: exp, PV, gate =================
            sc_box = [sc_prev]
            p_cmp_l = {}

            def emit_exp(b):
                if b in p_cmp_l:
                    return
                pc = pcmpp.tile([128, TRI], BF16, tag="p", name=f"pcmp{b}")
                ei = nc.scalar.activation(
                    out=pc, in_=d_cmp[b], func=AF.Exp, scale=w_col[:, 0:1]
                )
                sc_box[0] = _chain(sc_box[0], ei, "act-order")
                p_cmp_l[b] = pc

            p_lo_l = {}

            def get_plo(b):
                if b not in p_lo_l:
                    p_lo_l[b] = plop.tile([128, NLO], BF16, tag="plo", name=f"plo{b}")
                return p_lo_l[b]

            emitted_tr = set()

            def emit_transp(b, i):
                # transpose the missing lhsT tiles (k < i) for PV_i of batch b
                if (b, i) in emitted_tr:
                    return
                emitted_tr.add((b, i))
                p_cmp = p_cmp_l[b]
                p_lo = get_plo(b)
                psT = pstr.tile([128, 1024], BF16, tag="tr")
                for k in range(i):
                    nc.tensor.transpose(
                        psT[:, ts(k, 128)],
                        p_cmp[:, OFF[i] + 128 * k : OFF[i] + 128 * (k + 1)],
                        identb[:],
                    )
                dst = p_lo[:, LOFF[i] : LOFF[i] + 128 * i]
                if i <= 3:
                    ci = nc.scalar.copy(dst, psT[:, 0 : 128 * i])
                    sc_box[0] = _chain(sc_box[0], ci, "act-order")
                else:
                    nc.vector.tensor_copy(dst, psT[:, 0 : 128 * i])

            emit_exp(0)
            for b in range(BS):
                p_cmp = p_cmp_l[b]
                p_lo = get_plo(b)
                hw = hw_l[b]

                zs = None
                for i in range(NT):
                    if i + 1 < NT:
                        emit_transp(b, i + 1)
                    elif b + 1 < BS:
                        # lookahead: next batch's first transpose group + copy,
                        # so its p_lo copy lands before this batch's last tanh
                        # in the ScalarE chain (avoids the merged-wait stall).
                        emit_transp(b + 1, 1)
                    if i == 3 and b + 1 < BS:
                        emit_exp(b + 1)
                    if i % 4 == 0:
                        zs = ztp.tile([128, 4, A], F32, tag="zs", bufs=3)
                    pv = pspv.tile([128, 512], F32, tag="pv")
                    for k in range(NT):
                        if k >= i:
                            lhsT = p_cmp[:, OFF[k] + 128 * i : OFF[k] + 128 * (i + 1)]
                        else:
                            lhsT = p_lo[:, LOFF[i] + 128 * k : LOFF[i] + 128 * (k + 1)]
                        nc.tensor.matmul(
                            pv[:, 0 : A + 1],
                            lhsT,
                            hw[:, k, 0 : A + 1],
                            start=(k == 0),
                            stop=(k == NT - 1),
                        )
                    rp_i = smallp.tile([128, 1], F32, tag="rp_i")
                    nc.vector.reciprocal(rp_i[:], pv[:, A : A + 1])
                    nc.vector.scalar_tensor_tensor(
                        out=zs[:, i % 4, :],
                        in0=pv[:, 0:A],
                        scalar=rp_i[:, 0:1],
                        in1=hw[:, i, 257 : 257 + A],
                        op0=OP.mult,
                        op1=OP.add,
                    )
                    if i % 4 == 3:
                        g2 = i // 4
                        zo = ztp.tile([128, 4, A], F32, tag="zo")
                        ti = nc.scalar.activation(
                            out=zo[:].rearrange("p a b -> p (a b)"),
                            in_=zs[:].rearrange("p a b -> p (a b)"),
                            func=AF.Tanh,
                        )
                        sc_box[0] = _chain(sc_box[0], ti, "act-order")
                        for q in range(2):
                            r0 = 512 * g2 + 256 * q
                            nc.gpsimd.dma_start(
                                out=out_ext[r0 : r0 + 256, b, :].rearrange(
                                    "(u p) a -> p u a", p=128
                                ),
                                in_=zo[:, 2 * q : 2 * q + 2, :],
                            )
            stage2.__exit__(None, None, None)

    nc.compile()
    return nc


_CACHED = {}


def _get_graph():
    if "nc" not in _CACHED:
        _CACHED["nc"] = build_graph()
    return _CACHED["nc"]


def _run(inputs, trace=False, **kw):
    nc = _get_graph()
    x = np.asarray(inputs["x"], dtype=np.float32)
    h = np.asarray(inputs["h"], dtype=np.float32)
    w_sim = np.asarray(inputs["w_sim"], dtype=np.float32).reshape(1, 1)
    W_g = np.ascontiguousarray(np.asarray(inputs["W_g"], dtype=np.float32))
    b_g = np.asarray(inputs["b_g"], dtype=np.float32).reshape(1, A)
    in_maps = []
    for c in range(NCORES):
        in_maps.append(
            {
                "x": np.ascontiguousarray(x[:, c * BS : (c + 1) * BS, :]),
                "h": np.ascontiguousarray(h[:, c * BS : (c + 1) * BS, :]),
                "w_sim": w_sim,
                "W_g": W_g,
                "b_g": b_g,
            }
        )
    res = run_bass_kernel_spmd(nc, in_maps, list(range(NCORES)), trace=trace, **kw)
    out = np.concatenate([res.results[c]["out"] for c in range(NCORES)], axis=1)
    return out, res


def kernel(**inputs):
    out, _ = _run(inputs, trace=False)
    return out


if __name__ == "__main__":
    rng = np.random.default_rng(0)
    ins = {
        "x": rng.standard_normal((S, B, E), dtype=np.float32),
        "h": rng.standard_normal((S, B, H), dtype=np.float32),
        "w_sim": np.array([0.03], dtype=np.float32),
        "b_sim": np.array([0.01], dtype=np.float32),
        "W_g": (rng.standard_normal((A, 2 * H)) * 0.05).astype(np.float32),
        "b_g": np.zeros(A, dtype=np.float32),
    }
    out = kernel(**ins)
    print("out", out.shape, out.dtype, np.abs(out).mean())
